# revision 1
# baseline (speedup 1.0000x reference)
"""GRU decoder (teacher forcing) + log_softmax on 8 Trainium2 NeuronCores.

Strategy:
  - Vocab-shard the projection/log-softmax across the 8 cores (W_proj rows),
    replicate the (tiny, serial) GRU recurrence on every core.
  - Phase 0 (per 8-step chunk): indirect-DMA gather of embedding rows,
    DMA-xbar transpose to k-major, matmul -> x-side gate pre-activations
    XG = emb @ W_ih.T (+ b_ih + b_hh[r,z]) stored time-major in SBUF.
  - Phase 1 (63 sequential steps): hg^T = W_hh^T-slabs.T @ h^T on PE
    (weights stationary, batch on the moving free axis), gates on ACT/DVE
    in transposed [128, 4, 32] layout; h^T appended to HT (phase-2 input).
    sigmoid(x) = 0.5*tanh(x/2)+0.5 so only the exp_and_others ACT table is
    ever loaded (tanh+exp+identity live there; no table switches).
  - Phase 2 (16 row-tiles of 128): logits = HT-slabs.T @ W_projT-shard on
    PE; exp(logit - 4ln2) with accum_out gives row partial sums; raw logits
    are kept in fp16 SBUF rings. Per group of 4 row-tiles one tiny
    AllGather exchanges partial sums; lse is computed with a DVE
    bit-twiddle log (frexp + deg-5 poly) so ScalarE never switches tables;
    final out = logit - lse via ACT Identity with per-partition bias,
    then DMA to DRAM.
  - Scheduling: phase-1 instructions get the best (lowest) Tile priorities;
    phase-0 prep and phase-2 run in low-priority bands and fill PE/ACT/DVE
    idle slots of the serial recurrence.

kernel(**inputs) takes the FULL numpy inputs, does layout prep on host,
runs the SPMD NEFF on cores 0..7 and reassembles the [32, 64, 32000] output.
"""

import os

import numpy as np
import ml_dtypes

import concourse.bass as bass
import concourse.bacc as bacc
import concourse.mybir as mybir
import concourse.tile as tile
from concourse.bass_utils import run_bass_kernel_spmd
from concourse.masks import make_identity

# problem shape (hardcoded per contract)
B, T, V, E, H = 32, 64, 32000, 256, 512
S = T - 1                 # 63 decode steps
NCORES = 8
VS = V // NCORES          # 4000 vocab shard per core
G = 3 * H                 # 1536 gate dims
GC = G // 128             # 12 gate chunks
KH = H // 128             # 4 contraction tiles over H
KE = E // 128             # 2 contraction tiles over E
NROW = S * B              # 2016 output rows, (t, b) order
NMT = (NROW + 127) // 128  # 16 row-tiles (last has 96 rows)
NGRP = 8                  # stat-collective groups (2 row-tiles each)
MPG = NMT // NGRP         # 2 row-tiles per group
VU = 500                  # vocab unit for psum/exp
NVU = VS // VU            # 8 units per row-tile
LN2 = float(np.log(2.0))
EXP_BIAS = -4.0 * LN2     # exp(logit - 4ln2): keeps fp16 exp safely in range

F32 = mybir.dt.float32
BF16 = mybir.dt.bfloat16
F16 = mybir.dt.float16
I32 = mybir.dt.int32
U32 = mybir.dt.uint32
AF = mybir.ActivationFunctionType
OP = mybir.AluOpType

# -ln(m) Chebyshev-interpolation coefficients on m in [1, 2], highest first.
_nodes = np.cos((2 * np.arange(1, 7) - 1) / (2 * 6.0) * np.pi) * 0.5 + 1.5
_NEGLN_COEF = [float(c) for c in np.polyfit(_nodes, -np.log(_nodes), 5)]

_BUILD_CACHE = {}


def _build(bhh_n_nonzero: bool, bproj_nonzero: bool):
    debug = bool(int(os.environ.get("KERNEL_DEBUG", "0")))
    noprio = bool(int(os.environ.get("KERNEL_NOPRIO", "0")))
    key = (bhh_n_nonzero, bproj_nonzero, debug, noprio)
    if key in _BUILD_CACHE:
        return _BUILD_CACHE[key]

    nc = bacc.Bacc("TRN2", target_bir_lowering=False, debug=False,
                   enable_asserts=False, num_devices=NCORES)

    trg_d = nc.dram_tensor("trg_flat", (NROW, 1), I32, kind="ExternalInput")
    tbl_d = nc.dram_tensor("emb_tbl", (V, E), BF16, kind="ExternalInput")
    wih_d = nc.dram_tensor("wih_t", (128, KE, G), BF16, kind="ExternalInput")
    whh_d = nc.dram_tensor("whh_t", (128, KH, G), BF16, kind="ExternalInput")
    h0_d = nc.dram_tensor("h0_t", (128, KH, B), BF16, kind="ExternalInput")
    wpr_d = nc.dram_tensor("wproj_t", (128, KH, VS), BF16, kind="ExternalInput")
    bx_d = nc.dram_tensor("bx_t", (128, GC), BF16, kind="ExternalInput")
    if bhh_n_nonzero:
        bhn_d = nc.dram_tensor("bhn_t", (128, KH), BF16, kind="ExternalInput")
    if bproj_nonzero:
        bpr_d = nc.dram_tensor("bproj_s", (1, VS), F32, kind="ExternalInput")
    out_d = nc.dram_tensor("out_lp", (NROW, VS), F32, kind="ExternalOutput")
    if debug:
        ht_d = nc.dram_tensor("dbg_ht", (128, KH, NROW), BF16,
                              kind="ExternalOutput")
        xg_d = nc.dram_tensor("dbg_xg", (128, 8, GC, B), BF16,
                              kind="ExternalOutput")
        sall_d = nc.dram_tensor("dbg_sall", (128, NMT * NVU), F32,
                                kind="ExternalOutput")
        lg_d = nc.dram_tensor("dbg_lg", (128, VS), F16, kind="ExternalOutput")
        nlse_d = nc.dram_tensor("dbg_nlse", (128, MPG), F32,
                                kind="ExternalOutput")

    with tile.TileContext(nc) as tc:
        with tc.tile_pool(name="sb", bufs=1) as sb, \
             tc.tile_pool(name="ps", bufs=1, space="PSUM") as ps, \
             tc.tile_pool(name="dram", bufs=1, space="DRAM") as dp:

            # ---------- persistent loads / consts (highest priority band) ----
            wih_sb = sb.tile([128, KE, G], BF16)
            nc.sync.dma_start(wih_sb[:], wih_d[:])
            whh_sb = sb.tile([128, KH, G], BF16)
            nc.sync.dma_start(whh_sb[:], whh_d[:])
            wpr_sb = sb.tile([128, KH, VS], BF16)
            nc.sync.dma_start(wpr_sb[:], wpr_d[:])
            h0_sb = sb.tile([128, KH, B], BF16)
            nc.sync.dma_start(h0_sb[:], h0_d[:])
            bx_sb = sb.tile([128, GC], BF16)
            nc.sync.dma_start(bx_sb[:], bx_d[:])
            if bhh_n_nonzero:
                bhn_sb = sb.tile([128, KH], BF16)
                nc.sync.dma_start(bhn_sb[:], bhn_d[:])
            if bproj_nonzero:
                # materialize across partitions via 0-stride DMA broadcast
                bpr_sb = sb.tile([128, VS], F32)
                nc.gpsimd.dma_start(bpr_sb[:], bpr_d[:1, :].to_broadcast([128, VS]))

            HT = sb.tile([128, KH, NROW], BF16)      # h_{t+1} states, (t, b) cols
            ebias = sb.tile([128, 1], F32)
            nc.gpsimd.memset(ebias[:], EXP_BIAS)
            S_all = sb.tile([128, NMT * NVU], F32)   # exp partial sums
            nc.gpsimd.memset(S_all[:], 0.0)
            ident = sb.tile([128, 128], BF16)
            make_identity(nc, ident[:])
            # warm up the collective path (first AllGather pays ~25us extra)
            warm_in = dp.tile([128, 1], F32, tag="warm_in")
            warm_out = dp.tile([NCORES, 128, 1], F32, tag="warm_out",
                               addr_space="Shared")
            nc.gpsimd.dma_start(warm_in[:], ebias[:])
            nc.gpsimd.collective_compute(
                "AllGather", OP.bypass, replica_groups=[list(range(NCORES))],
                ins=[warm_in.opt()], outs=[warm_out.opt()])

            # ---------------- phase 0: one chunk of XG prep -----------------
            xg_tiles = {}

            def emit_prep_gather(c8):
                tlo = 8 * c8
                nst = min(8, S - tlo)
                nrows = B * nst
                xg = sb.tile([128, 8, GC, B], BF16, tag="xg", bufs=2,
                             name=f"xg{c8}")
                xg_tiles[c8] = xg
                embt = sb.tile([128, KE, 256], BF16, tag="embt", bufs=2,
                               name=f"embt{c8}")
                for sub in range(2):
                    lo = tlo * B + sub * 128
                    nr = min(128, nrows - sub * 128)
                    if nr <= 0:
                        continue
                    idx_t = sb.tile([128, 1], I32, tag="idx", bufs=4,
                                    name=f"idx{c8}_{sub}")
                    nc.sync.dma_start(idx_t[:nr], trg_d[lo:lo + nr, :])
                    rows = sb.tile([128, E], BF16, tag="embr", bufs=4,
                                   name=f"embr{c8}_{sub}")
                    nc.gpsimd.indirect_dma_start(
                        out=rows[:nr], out_offset=None, in_=tbl_d[:],
                        in_offset=bass.IndirectOffsetOnAxis(ap=idx_t[:nr, :1], axis=0))
                    for kb in range(KE):
                        nc.sync.dma_start_transpose(
                            embt[:, kb, sub * 128:sub * 128 + nr],
                            rows[:nr, kb * 128:(kb + 1) * 128])
                return embt

            def emit_prep_xg(c8, embt, gcs):
                tlo = 8 * c8
                nst = min(8, S - tlo)
                nrows = B * nst
                xg = xg_tiles[c8]
                for gc in gcs:
                    pxg = ps.tile([128, 256], F32, tag="ps_xg", bufs=1,
                                  name=f"pxg{c8}_{gc}")
                    for kt in range(KE):
                        nc.tensor.matmul(
                            pxg[:, :nrows],
                            lhsT=wih_sb[:, kt, gc * 128:(gc + 1) * 128],
                            rhs=embt[:, kt, :nrows],
                            start=(kt == 0), stop=(kt == KE - 1))
                    # xg[:, t, gc, :] = pxg + (b_ih + b_hh[r,z])[gc broadcast]
                    nc.vector.tensor_tensor(
                        out=xg[:, :nst, gc, :],
                        in0=pxg[:, :nrows].rearrange("p (t b) -> p t b", b=B),
                        in1=bx_sb[:, gc:gc + 1].to_broadcast([128, nst, B]),
                        op=OP.add)

            def emit_prep(c8):
                embt = emit_prep_gather(c8)
                emit_prep_xg(c8, embt, range(GC))

            emit_prep(0)
            if debug:
                nc.sync.dma_start(xg_d[:], xg_tiles[0][:])

            # ---------------- phase 1 + interleaved emission ----------------
            r_off, z_off, n_off = 0, 4, 8   # gate chunk offsets (r, z, n)

            def emit_step(t):
                h_prev = h0_sb[:, :, :] if t == 0 else HT[:, :, (t - 1) * B:t * B]
                xg = xg_tiles[t // 8][:, t % 8, :, :]
                ps_r = ps.tile([128, 4, B], F32, tag="ps_r", name=f"psr{t}")
                ps_n = ps.tile([128, 4, B], F32, tag="ps_n", name=f"psn{t}")
                ps_z = ps.tile([128, 4, B], F32, tag="ps_z", name=f"psz{t}")
                # W_hh matmuls + fold the x-side gate pre-acts (xr, xz) into
                # PSUM with an extra identity matmul (start=False accumulate).
                for dst, off in ((ps_r, r_off), (ps_z, z_off), (ps_n, n_off)):
                    fold_x = off in (r_off, z_off)
                    for gc in range(4):
                        for kt in range(KH):
                            nc.tensor.matmul(
                                dst[:, gc, :],
                                lhsT=whh_sb[:, kt, (off + gc) * 128:(off + gc + 1) * 128],
                                rhs=h_prev[:, kt, :],
                                start=(kt == 0),
                                stop=(not fold_x and kt == KH - 1))
                    if fold_x:
                        for gc in range(4):
                            nc.tensor.matmul(
                                dst[:, gc, :], lhsT=ident[:],
                                rhs=xg[:, off + gc, :],
                                start=False, stop=True)
                # r gate: sigma(x) = 0.5*tanh(x/2) + 0.5
                rt = sb.tile([128, 4, B], BF16, tag="rt", bufs=2, name=f"rt{t}")
                nc.scalar.activation(rt[:], ps_r[:], AF.Tanh, scale=0.5)
                r_s = sb.tile([128, 4, B], BF16, tag="r_s", bufs=2, name=f"rs{t}")
                nc.vector.tensor_scalar(out=r_s[:], in0=rt[:], scalar1=0.5,
                                        scalar2=0.5, op0=OP.mult, op1=OP.add)
                # z gate -> zt = tanh(z_pre/2); q = 1-z, p = z*h on GpSimd
                zt = sb.tile([128, 4, B], BF16, tag="zt", bufs=2, name=f"zt{t}")
                nc.scalar.activation(zt[:], ps_z[:], AF.Tanh, scale=0.5)
                q_s = sb.tile([128, 4, B], BF16, tag="q_s", bufs=2, name=f"qs{t}")
                nc.gpsimd.tensor_scalar(out=q_s[:], in0=zt[:], scalar1=-0.5,
                                        scalar2=0.5, op0=OP.mult, op1=OP.add)
                z_s = sb.tile([128, 4, B], BF16, tag="z_s", bufs=2, name=f"zs{t}")
                nc.gpsimd.tensor_scalar(out=z_s[:], in0=zt[:], scalar1=0.5,
                                        scalar2=0.5, op0=OP.mult, op1=OP.add)
                p_s = sb.tile([128, 4, B], BF16, tag="p_s", bufs=2, name=f"ps{t}")
                nc.gpsimd.tensor_tensor(out=p_s[:], in0=z_s[:], in1=h_prev,
                                        op=OP.mult)
                # n gate
                if bhh_n_nonzero:
                    nc.vector.tensor_tensor(
                        out=ps_n[:], in0=ps_n[:],
                        in1=bhn_sb[:, :, None].to_broadcast([128, 4, B]), op=OP.add)
                nc.vector.tensor_tensor(out=ps_n[:], in0=ps_n[:], in1=r_s[:],
                                        op=OP.mult)
                nc.vector.tensor_tensor(out=ps_n[:], in0=ps_n[:],
                                        in1=xg[:, n_off:n_off + 4, :], op=OP.add)
                n_s = sb.tile([128, 4, B], BF16, tag="n_s", bufs=2, name=f"ns{t}")
                nc.scalar.activation(n_s[:], ps_n[:], AF.Tanh)
                # h' = n*(1-z) + z*h
                w_s = sb.tile([128, 4, B], BF16, tag="w_s", bufs=2, name=f"ws{t}")
                nc.vector.tensor_tensor(out=w_s[:], in0=n_s[:], in1=q_s[:],
                                        op=OP.mult)
                nc.vector.tensor_tensor(out=HT[:, :, t * B:(t + 1) * B],
                                        in0=w_s[:], in1=p_s[:], op=OP.add)

            # ---------------- phase 2 emission helpers ----------------------
            logit_tiles = {}
            lse_tiles = {}

            def emit_munit(m, u):
                # one 500-vocab unit of row-tile m's logits + exp stats
                mp = min(128, NROW - m * 128)
                if u == 0:
                    logit_tiles[m] = sb.tile([128, VS], F16, tag="logit",
                                             bufs=6, name=f"lg{m}")
                lg = logit_tiles[m]
                pl = ps.tile([128, 500], F32, tag="ps_l", bufs=3,
                             name=f"pl{m}_{u}")
                for kt in range(KH):
                    nc.tensor.matmul(
                        pl[:mp], lhsT=HT[:, kt, m * 128:m * 128 + mp],
                        rhs=wpr_sb[:, kt, u * 500:(u + 1) * 500],
                        start=(kt == 0), stop=(kt == KH - 1))
                if bproj_nonzero:
                    nc.vector.tensor_tensor(out=pl[:mp], in0=pl[:mp],
                                            in1=bpr_sb[:mp, u * 500:(u + 1) * 500],
                                            op=OP.add)
                nc.vector.tensor_copy(lg[:mp, u * 500:(u + 1) * 500], pl[:mp])
                esc = sb.tile([128, 500], F16, tag="exps", bufs=2,
                              name=f"esc{m}_{u}")
                nc.scalar.activation(esc[:mp], pl[:mp], AF.Exp,
                                     bias=ebias[:mp, :1], scale=1.0,
                                     accum_out=S_all[:mp, m * NVU + u:m * NVU + u + 1])

            def emit_group_stats(g):
                # local row-sums for the group's 4 row-tiles
                sg = sb.tile([128, MPG], F32, tag="sg", bufs=2, name=f"sg{g}")
                for j in range(MPG):
                    m = g * MPG + j
                    nc.vector.reduce_sum(
                        out=sg[:, j:j + 1],
                        in_=S_all[:, m * NVU:(m + 1) * NVU],
                        axis=mybir.AxisListType.X)
                cin = dp.tile([128, MPG], F32, tag=f"cin{g}", name=f"cin{g}")
                nc.gpsimd.dma_start(cin[:], sg[:])
                cout = dp.tile([NCORES, 128, MPG], F32, tag=f"cout{g}",
                               addr_space="Shared", name=f"cout{g}")
                nc.gpsimd.collective_compute(
                    "AllGather", OP.bypass,
                    replica_groups=[list(range(NCORES))],
                    ins=[cin.opt()], outs=[cout.opt()])
                s8 = sb.tile([128, MPG, NCORES], F32, tag="s8", bufs=2,
                             name=f"s8{g}")
                nc.gpsimd.dma_start(s8[:], cout[:].rearrange("c p m -> p m c"))
                st = sb.tile([128, MPG], F32, tag="st", bufs=2, name=f"st{g}")
                nc.vector.reduce_sum(out=st[:], in_=s8[:],
                                     axis=mybir.AxisListType.X)
                # neg_lse = -(e - 127 + 4) * ln2 - ln(m),  St = m * 2^(e-127)
                # (the exp bias -4ln2 shifts lse by -4ln2; fold via e+4... see below)
                iu = st[:].bitcast(U32)
                eu = sb.tile([128, MPG], U32, tag="eu", bufs=2, name=f"eu{g}")
                nc.vector.tensor_scalar(out=eu[:], in0=iu, scalar1=23,
                                        scalar2=None, op0=OP.logical_shift_right)
                ef = sb.tile([128, MPG], F32, tag="ef", bufs=2, name=f"ef{g}")
                nc.vector.tensor_copy(ef[:], eu[:])
                mu = sb.tile([128, MPG], U32, tag="mu", bufs=2, name=f"mu{g}")
                nc.vector.tensor_scalar(out=mu[:], in0=iu, scalar1=0x007FFFFF,
                                        scalar2=0x3F800000, op0=OP.bitwise_and,
                                        op1=OP.bitwise_or)
                mf = mu[:].bitcast(F32)
                acc = sb.tile([128, MPG], F32, tag="acc", bufs=2, name=f"acc{g}")
                c = _NEGLN_COEF
                nc.vector.tensor_scalar(out=acc[:], in0=mf, scalar1=c[0],
                                        scalar2=c[1], op0=OP.mult, op1=OP.add)
                for k in range(2, 6):
                    nc.vector.tensor_tensor(out=acc[:], in0=acc[:], in1=mf,
                                            op=OP.mult)
                    nc.vector.tensor_scalar(out=acc[:], in0=acc[:], scalar1=c[k],
                                            scalar2=None, op0=OP.add)
                # + (127 - 4 - e) * ln2   (the -4 re-adds the exp bias so
                #   lse refers to unshifted logits)
                e2 = sb.tile([128, MPG], F32, tag="e2", bufs=2, name=f"e2{g}")
                nc.vector.tensor_scalar(out=e2[:], in0=ef[:], scalar1=-LN2,
                                        scalar2=(127.0 - 4.0) * LN2,
                                        op0=OP.mult, op1=OP.add)
                nlse = sb.tile([128, MPG], F32, tag="nlse", bufs=2,
                               name=f"nlse{g}")
                nc.vector.tensor_tensor(out=nlse[:], in0=acc[:], in1=e2[:],
                                        op=OP.add)
                lse_tiles[g] = nlse
                if debug and g == 0:
                    nc.sync.dma_start(nlse_d[:], nlse[:])

            def emit_output(m):
                g, j = m // MPG, m % MPG
                mp = min(128, NROW - m * 128)
                nlse = lse_tiles[g]
                lg = logit_tiles.pop(m)
                if debug and m == 0:
                    nc.sync.dma_start(lg_d[:], lg[:])
                ot = sb.tile([128, VS], F32, tag="ot", bufs=2, name=f"ot{m}")
                nc.scalar.activation(ot[:mp], lg[:mp], AF.Identity,
                                     bias=nlse[:mp, j:j + 1])
                nc.sync.dma_start(out_d[m * 128:m * 128 + mp, :], ot[:mp])

            # ---------------- main emission loop ----------------------------
            # Interleave prep / phase-2 work between steps in small pieces so
            # the scheduler can't starve the serial recurrence on PE.
            from collections import deque
            work_q = deque()

            def enqueue_mtile(m):
                for u in range(NVU):
                    work_q.append(lambda m=m, u=u: emit_munit(m, u))
                if m % MPG == MPG - 1:
                    g = m // MPG

                    def fin(g=g):
                        emit_group_stats(g)
                        for mm in range(g * MPG, (g + 1) * MPG):
                            emit_output(mm)
                    work_q.append(fin)

            for t in range(S):
                emit_step(t)
                if t % 8 == 1 and t // 8 + 1 <= (S - 1) // 8:
                    c8 = t // 8 + 1
                    embt = emit_prep_gather(c8)
                    for lo in range(0, GC, 3):
                        work_q.append(lambda c8=c8, embt=embt, lo=lo:
                                      emit_prep_xg(c8, embt, range(lo, min(lo + 3, GC))))
                if t >= 3 and (t - 3) % 4 == 0:
                    enqueue_mtile((t - 3) // 4)
                ndrain = 3 if t < S - 1 else len(work_q)
                for _ in range(min(ndrain, len(work_q))):
                    work_q.popleft()()
            for m in range(((S - 1 - 3) // 4) + 1, NMT):
                enqueue_mtile(m)
            while work_q:
                work_q.popleft()()
            if debug:
                nc.sync.dma_start(ht_d[:], HT[:])
                nc.sync.dma_start(sall_d[:], S_all[:])

    nc.finalize()
    _BUILD_CACHE[key] = nc
    return nc


def _pack_T(w, ktiles):
    """[out_dim, in_dim] f32 -> [128, ktiles, out_dim] bf16 (w.T, k-major slabs)."""
    wT = np.ascontiguousarray(w.T).astype(ml_dtypes.bfloat16)
    return np.ascontiguousarray(
        wT.reshape(ktiles, 128, w.shape[0]).transpose(1, 0, 2))


LAST_PROFILE = None


def kernel(trg, h0, embed_table, W_ih, W_hh, b_ih, b_hh, W_proj, b_proj):
    global LAST_PROFILE
    trg = np.asarray(trg)
    h0 = np.asarray(h0, dtype=np.float32)
    embed_table = np.asarray(embed_table, dtype=np.float32)
    W_ih = np.asarray(W_ih, dtype=np.float32)
    W_hh = np.asarray(W_hh, dtype=np.float32)
    b_ih = np.asarray(b_ih, dtype=np.float32)
    b_hh = np.asarray(b_hh, dtype=np.float32)
    W_proj = np.asarray(W_proj, dtype=np.float32)
    b_proj = np.asarray(b_proj, dtype=np.float32)

    bhh_n_nonzero = bool(np.any(b_hh[2 * H:]))
    bproj_nonzero = bool(np.any(b_proj))
    nc = _build(bhh_n_nonzero, bproj_nonzero)

    # host-side layout prep (sharding/packing only)
    trg_flat = np.ascontiguousarray(
        trg[:, :S].T.reshape(NROW, 1)).astype(np.int32)
    tbl_bf = embed_table.astype(ml_dtypes.bfloat16)
    wih_t = _pack_T(W_ih, KE)
    whh_t = _pack_T(W_hh, KH)
    h0_t = np.ascontiguousarray(
        h0[0].T.reshape(KH, 128, B).transpose(1, 0, 2)).astype(ml_dtypes.bfloat16)
    # bx = b_ih + [b_hh for r,z chunks; 0 for n chunks], packed [128, GC]
    bx = b_ih.copy()
    bx[:2 * H] += b_hh[:2 * H]
    bx_t = np.ascontiguousarray(bx.reshape(GC, 128).T).astype(ml_dtypes.bfloat16)

    base = {
        "trg_flat": trg_flat,
        "emb_tbl": tbl_bf,
        "wih_t": wih_t,
        "whh_t": whh_t,
        "h0_t": h0_t,
        "bx_t": bx_t,
    }
    if bhh_n_nonzero:
        base["bhn_t"] = np.ascontiguousarray(
            b_hh[2 * H:].reshape(KH, 128).T).astype(ml_dtypes.bfloat16)

    in_maps = []
    for c in range(NCORES):
        m = dict(base)
        m["wproj_t"] = _pack_T(W_proj[c * VS:(c + 1) * VS], KH)
        if bproj_nonzero:
            m["bproj_s"] = np.ascontiguousarray(
                b_proj[c * VS:(c + 1) * VS].reshape(1, VS))
        in_maps.append(m)

    trace = bool(int(os.environ.get("KERNEL_TRACE", "0")))
    res = run_bass_kernel_spmd(nc, in_maps, core_ids=list(range(NCORES)),
                               trace=trace)
    LAST_PROFILE = res

    out = np.zeros((B, T, V), dtype=np.float32)
    big = np.stack([res.results[c]["out_lp"].reshape(S, B, VS)
                    for c in range(NCORES)], axis=0)   # [c, t, b, vs]
    out[:, 1:, :] = big.transpose(2, 1, 0, 3).reshape(B, S, V)
    return out

